# revision 13
# baseline (speedup 1.0000x reference)
"""Trainium2 Bass kernel for LoRACrossAttnProcessor (v4, bf16, PE-continuity).

Strategy (changes vs v3):
- Q-proj(0) is the FIRST PE phase: it can start once x(b0,st0) + the
  first 256-col wq chunk land (~2MB) instead of waiting for wk+et+wq+x
  (~6MB).  K proj runs after Q0, its weights stream in under Q0's 46us
  of PE work with zero stall.
- Softmax normalization is applied to exps BEFORE the AV matmuls:
  sum-exp (PE) -> recip (DVE, f32) -> bf16 row -> partition-broadcast
  to 77 rows (GPSIMD) -> in-place bf16 multiply (DVE fast mode).  The
  whole chain runs under q_proj(1)/o_proj(0) PE cover.  AV evacuation
  is then a plain PSUM->SBUF cast, alternating DVE/ACT, with PSUM
  banks alternating psV/psSE, so the AV phase never starves.
- scores/softmax phases hoisted ahead of the big GEMMs; within a
  phase all 8 heads' score matmuls run before the 8 sum-exp matmuls.
  PE stalls are doubly expensive on TRN2: the PE clock drops to
  1.2GHz after an idle and needs 3us of continuous work to re-ramp.
- V projection over plain 1280 cols with scatter-evac into the
  zero-padded (head,tile)-pair layout.
- Fine-grained DMAs issued in consumption order; o_proj output DMA
  per 512-col chunk; evacuations alternate DVE/ACT.

Numerics: host folds LoRA exactly (f64), everything bf16 on device,
f32 PSUM.  (fp8 DoubleRow was measured at 2x bf16 MACs -- the 3-term
error-compensated scheme would be 1.4x slower than bf16, so not used.)
"""

import numpy as np
from contextlib import ExitStack

import ml_dtypes

import concourse.bass as bass
import concourse.mybir as mybir
import concourse.tile as tile
from concourse import bacc, bass_isa
from concourse.bass_utils import run_bass_kernel_spmd

F32 = mybir.dt.float32
BF16 = mybir.dt.bfloat16
AF = mybir.ActivationFunctionType
MULT = mybir.AluOpType.mult

H = 8
B, S, C = 16, 1024, 1280
SENC, CENC = 77, 1024
D = C // H  # 160
NCORES = 8
BPC = B // NCORES  # 2 batches per core
P = 128
NCI_Q = C // P  # 10 contraction tiles for Q/O proj
NCI_KV = CENC // P  # 8 contraction tiles for K/V proj
NCO = C // P  # 10 output-channel tiles
NST = S // 512  # 2 seq chunks of 512
SENC2 = 2 * SENC  # 154
ATTN_SCALE = 1.0 / float(np.sqrt(D))
OCHUNKS = [(0, 512), (512, 512), (1024, 256)]

# (head, tile) pairs: head h covers channels [160h, 160h+160); tile t covers
# [128t, 128t+128). Each pair gets one 128-col slot in the vm layout.
PAIRS = []
for _h in range(H):
    for _t in range(NCO):
        lo = max(D * _h, P * _t)
        hi = min(D * _h + D, P * _t + P)
        if lo < hi:
            PAIRS.append((_h, _t, lo, hi))
NPAIR = len(PAIRS)  # 16
PAIRS_OF_TILE = {t: [i for i, p in enumerate(PAIRS) if p[1] == t] for t in range(NCO)}
TILES_OF_HEAD = {h: sorted({p[1] for p in PAIRS if p[0] == h}) for h in range(H)}


def aligned_ranges(r0, r1):
    """Decompose [r0, r1) (within one 128 tile) into blocks of size 32/64/128
    with offset % size == 0 (SBUF partition-access alignment rule)."""
    out = []
    g = r0
    while g < r1:
        s = 128
        while s > r1 - g or g % s != 0:
            s //= 2
        out.append((g, s))
        g += s
    return out


def build():
    nc = bacc.Bacc("TRN2", target_bir_lowering=False, debug=False)
    xt_d = nc.dram_tensor("xt", [BPC, C, S], BF16, kind="ExternalInput")
    et_d = nc.dram_tensor("et", [CENC, SENC2], BF16, kind="ExternalInput")
    wqt_d = nc.dram_tensor("wqt", [C, C], BF16, kind="ExternalInput")
    wkt_d = nc.dram_tensor("wkt", [CENC, C], BF16, kind="ExternalInput")
    wvt_d = nc.dram_tensor("wvt", [CENC, C], BF16, kind="ExternalInput")
    wot_d = nc.dram_tensor("wot", [C, C], BF16, kind="ExternalInput")
    out_d = nc.dram_tensor("out", [BPC, S, C], BF16, kind="ExternalOutput")

    with tile.TileContext(nc) as tc, ExitStack() as ctx:
        big = ctx.enter_context(tc.tile_pool(name="big", bufs=4))
        wpool = ctx.enter_context(tc.tile_pool(name="wpool", bufs=1))
        persist = ctx.enter_context(tc.tile_pool(name="persist", bufs=1))
        expnp = ctx.enter_context(tc.tile_pool(name="expnp", bufs=3))
        smallp = ctx.enter_context(tc.tile_pool(name="smallp", bufs=4))
        stag = ctx.enter_context(tc.tile_pool(name="stag", bufs=2))
        psA = ctx.enter_context(tc.tile_pool(name="psA", bufs=2, space="PSUM"))
        psSE = ctx.enter_context(tc.tile_pool(name="psSE", bufs=3, space="PSUM"))
        psV = ctx.enter_context(tc.tile_pool(name="psV", bufs=3, space="PSUM"))

        # ---- DMAs in strict consumption order ----
        # Q0 feed first: x(b0,st0) per-ci + wq in 256-col chunks.
        x_s = [None] * BPC
        for b in range(BPC):
            x_s[b] = big.tile([P, NCI_Q, S], BF16, tag="big", name=f"xt{b}")
        wq_s = wpool.tile([P, NCI_Q, C], BF16, tag="wB", name="wq_s")

        def dma_x(b, st):
            sl = slice(st * 512, st * 512 + 512)
            for ci in range(NCI_Q):
                nc.sync.dma_start(
                    out=x_s[b][:, ci, sl],
                    in_=xt_d.ap()[b, ci * P : (ci + 1) * P, sl],
                )

        # first Q matmul needs only wq cols 0:128 + x(b0,st0,ci=0): 0.46MB
        def dma_wq(ch, cn):
            nc.sync.dma_start(
                out=wq_s[:, :, ch : ch + cn],
                in_=wqt_d.ap()[:, ch : ch + cn].rearrange(
                    "(ci p) c -> p ci c", p=P
                ),
            )

        def dma_x1(b, st, cis):
            sl = slice(st * 512, st * 512 + 512)
            for ci in cis:
                nc.sync.dma_start(
                    out=x_s[b][:, ci, sl],
                    in_=xt_d.ap()[b, ci * P : (ci + 1) * P, sl],
                )

        dma_wq(0, 128)
        dma_x1(0, 0, range(0, 7))
        dma_wq(128, 256)
        dma_x1(0, 0, range(7, 10))
        dma_wq(384, 256)
        dma_x1(0, 1, range(0, 5))
        dma_wq(640, 256)
        dma_x1(0, 1, range(5, 10))
        dma_wq(896, 256)
        dma_wq(1152, 128)

        # K feed (consumed ~+46us), then x(b1) (Q1 ~+60us), wv (V ~+105us),
        # wo (O0 ~+135us; shares wk's slot -> also waits on K proj reads).
        et_s = persist.tile([P, NCI_KV, SENC2], BF16, tag="et")
        wk_s = wpool.tile([P, NCI_KV, C], BF16, tag="wA")
        for ci in range(NCI_KV):
            nc.sync.dma_start(
                out=et_s[:, ci, :], in_=et_d.ap()[ci * P : (ci + 1) * P, :]
            )
            nc.sync.dma_start(
                out=wk_s[:, ci, :], in_=wkt_d.ap()[ci * P : (ci + 1) * P, :]
            )
        ones77 = persist.tile([SENC, 1], BF16, tag="ones77")
        nc.vector.memset(ones77, 1.0)
        dma_x(1, 0)
        dma_x(1, 1)
        wv_s = wpool.tile([P, NCI_KV, C], BF16, tag="wC", name="wv_s")
        for ci in range(NCI_KV):
            nc.sync.dma_start(
                out=wv_s[:, ci, :], in_=wvt_d.ap()[ci * P : (ci + 1) * P, :]
            )

        qt = [None] * BPC
        at = [None] * BPC
        vm = [None] * BPC
        kt_r = []
        exps_w = {}

        def q_proj(b, tiles):
            if qt[b] is None:
                qt[b] = big.tile([P, NCO, S], BF16, tag="big", name=f"qt{b}")
            for st, co in tiles:
                    sl = slice(st * 512, st * 512 + 512)
                    ps = psA.tile([P, 512], F32, tag="ps")
                    for ci in range(NCI_Q):
                        nc.tensor.matmul(
                            ps,
                            wq_s[:, ci, co * P : (co + 1) * P],
                            x_s[b][:, ci, sl],
                            start=(ci == 0),
                            stop=(ci == NCI_Q - 1),
                        )
                    nc.scalar.copy(out=qt[b][:, co, sl], in_=ps)

        def k_proj():
            for t in range(NCO):
                ps = psA.tile([P, 512], F32, tag="ps")
                for ci in range(NCI_KV):
                    nc.tensor.matmul(
                        ps[:, :SENC2],
                        wk_s[:, ci, t * P : (t + 1) * P],
                        et_s[:, ci, :],
                        start=(ci == 0),
                        stop=(ci == NCI_KV - 1),
                    )
                kte = persist.tile([P, SENC2], BF16, tag=f"kte{t}", name=f"kte{t}")
                kto = persist.tile([P, SENC2], BF16, tag=f"kto{t}", name=f"kto{t}")
                nc.vector.memset(kte, 0.0)
                nc.vector.memset(kto, 0.0)
                for h in range(H):
                    r0 = max(D * h, P * t)
                    r1 = min(D * h + D, P * t + P)
                    if r0 >= r1:
                        continue
                    dst = kte if h % 2 == 0 else kto
                    for o, sz in aligned_ranges(r0 - P * t, r1 - P * t):
                        nc.vector.tensor_copy(
                            out=dst[o : o + sz, :], in_=ps[o : o + sz, :SENC2]
                        )
                kt_r.append((kte, kto))

        def v_proj(b):
            # plain V proj (1280 cols in 3 chunks), scatter-evac into the
            # zero-padded (head,tile)-pair layout vm[b] [77, 16, 128]
            vm[b] = persist.tile([SENC, NPAIR, P], BF16, tag=f"vm{b}", name=f"vm{b}")
            for pi, (h, t, lo, hi) in enumerate(PAIRS):
                a, z = lo - P * t, hi - P * t
                if a > 0:
                    nc.gpsimd.memset(vm[b][:, pi, 0:a], 0.0)
                if z < P:
                    nc.gpsimd.memset(vm[b][:, pi, z:P], 0.0)
            bsl = slice(b * SENC, (b + 1) * SENC)
            k = 0
            for j, (c0, cn) in enumerate(OCHUNKS):
                ps = psV.tile([P, 512], F32, tag="ps")
                for ci in range(NCI_KV):
                    nc.tensor.matmul(
                        ps[:SENC, :cn],
                        et_s[:, ci, bsl],
                        wv_s[:, ci, c0 : c0 + cn],
                        start=(ci == 0),
                        stop=(ci == NCI_KV - 1),
                    )
                for pi, (h, t, lo, hi) in enumerate(PAIRS):
                    if lo >= c0 + cn or hi <= c0:
                        continue
                    nc.vector.tensor_copy(
                        out=vm[b][:, pi, lo - P * t : hi - P * t],
                        in_=ps[:SENC, lo - c0 : hi - c0],
                    )

        def sc_part(b, st):
            # scores + exp for all 8 heads of one 512-query window
            bsl = slice(b * SENC, (b + 1) * SENC)
            sl = slice(st * 512, st * 512 + 512)
            exps = expnp.tile([SENC, H, 512], BF16, tag="expn")
            exps_w[(b, st)] = exps
            for h in range(H):
                tiles = TILES_OF_HEAD[h]
                ps_s = psSE.tile([SENC, 512], F32, tag="ps")
                for i, t in enumerate(tiles):
                    nc.tensor.matmul(
                        ps_s,
                        kt_r[t][h % 2][:, bsl],
                        qt[b][:, t, sl],
                        start=(i == 0),
                        stop=(i == len(tiles) - 1),
                    )
                nc.scalar.activation(
                    out=exps[:, h, :], in_=ps_s, func=AF.Exp, scale=ATTN_SCALE
                )

        def se_part(b, st):
            # per head: sum-exp (PE) -> recip (DVE f32) -> bf16 row ->
            # partition-broadcast (GPSIMD) -> in-place exps *= 1/Z (DVE).
            # All normalize work drains under following q/o_proj PE cover.
            exps = exps_w[(b, st)]
            for h in range(H):
                ps_se = psA.tile([P, 512], F32, tag="ps", name="ps_se")
                nc.tensor.matmul(
                    ps_se[0:1, :], ones77, exps[:, h, :], start=True, stop=True
                )
                nc.vector.reciprocal_approx_fast(
                    out=ps_se[0:1, :], in_=ps_se[0:1, :]
                )
                recb = smallp.tile([1, 512], BF16, tag="recb")
                nc.vector.tensor_copy(out=recb, in_=ps_se[0:1, :])
                zb = smallp.tile([SENC, 512], BF16, tag="zb")
                nc.gpsimd.partition_broadcast(zb, recb)
                nc.vector.tensor_tensor(
                    out=exps[:, h, :], in0=exps[:, h, :], in1=zb, op=MULT
                )

        def av_phase(b, st):
            if at[b] is None:
                at[b] = big.tile([P, NCO, S], BF16, tag="big", name=f"at{b}")
            sl = slice(st * 512, st * 512 + 512)
            exps = exps_w.pop((b, st))
            for t in range(NCO):
                pairs = PAIRS_OF_TILE[t]
                ps_av = psV.tile([P, 512], F32, tag="ps")
                for i, pi in enumerate(pairs):
                    ph = PAIRS[pi][0]
                    nc.tensor.matmul(
                        ps_av,
                        vm[b][:, pi, :],
                        exps[:, ph, :],
                        start=(i == 0),
                        stop=(i == len(pairs) - 1),
                    )
                if t % 2 == 0:
                    nc.vector.tensor_copy(out=at[b][:, t, sl], in_=ps_av)
                else:
                    nc.scalar.copy(out=at[b][:, t, sl], in_=ps_av)

        def o_proj(b, stiles):
            for stile in stiles:
                s0 = stile * P
                ost = stag.tile([P, C], BF16, tag="ost")
                for j, (c0, cn) in enumerate(OCHUNKS):
                    pso = psA.tile([P, 512], F32, tag="ps", name="pso")
                    for ci in range(NCI_Q):
                        nc.tensor.matmul(
                            pso[:, :cn],
                            at[b][:, ci, s0 : s0 + P],
                            wo_s[:, ci, c0 : c0 + cn],
                            start=(ci == 0),
                            stop=(ci == NCI_Q - 1),
                        )
                    if j % 2 == 0:
                        nc.vector.tensor_copy(
                            out=ost[:, c0 : c0 + cn], in_=pso[:, :cn]
                        )
                    else:
                        nc.scalar.copy(out=ost[:, c0 : c0 + cn], in_=pso[:, :cn])
                    nc.sync.dma_start(
                        out=out_d.ap()[b, s0 : s0 + P, c0 : c0 + cn],
                        in_=ost[:, c0 : c0 + cn],
                    )

        QTILES = [(st, co) for st in range(NST) for co in range(NCO)]
        # PE issue order = PE execution order (in-order engine queues).
        q_proj(0, QTILES)
        k_proj()
        # wo reuses wk's slot; issue after k_proj so the WAR dep is clean.
        wo_s = wpool.tile([P, NCI_Q, C], BF16, tag="wA", name="wo_s")
        for ci in range(NCI_Q):
            nc.sync.dma_start(
                out=wo_s[:, ci, :], in_=wot_d.ap()[ci * P : (ci + 1) * P, :]
            )
        sc_part(0, 0)
        sc_part(0, 1)
        se_part(0, 0)
        se_part(0, 1)
        q_proj(1, QTILES)
        v_proj(0)
        v_proj(1)
        av_phase(0, 0)
        av_phase(0, 1)
        sc_part(1, 0)
        sc_part(1, 1)
        se_part(1, 0)
        se_part(1, 1)
        o_proj(0, range(8))
        av_phase(1, 0)
        av_phase(1, 1)
        o_proj(1, range(8))

    nc.compile()
    return nc


_NC_CACHE = []


def _get_nc():
    if not _NC_CACHE:
        _NC_CACHE.append(build())
    return _NC_CACHE[0]


def make_in_maps(hidden_states, encoder_hidden_states, Wq, Wk, Wv, Wo,
                 q_down, q_up, k_down, k_up, v_down, v_up, o_down, o_up):
    bf = ml_dtypes.bfloat16
    wq = (Wq.astype(np.float64) + q_up.astype(np.float64) @ q_down.astype(np.float64))
    wk = (Wk.astype(np.float64) + k_up.astype(np.float64) @ k_down.astype(np.float64))
    wv = (Wv.astype(np.float64) + v_up.astype(np.float64) @ v_down.astype(np.float64))
    wo = (Wo.astype(np.float64) + o_up.astype(np.float64) @ o_down.astype(np.float64))
    wqt = np.ascontiguousarray(wq.T).astype(bf)
    wkt = np.ascontiguousarray(wk.T).astype(bf)
    wvt = np.ascontiguousarray(wv.T).astype(bf)
    wot = np.ascontiguousarray(wo.T).astype(bf)

    in_maps = []
    for c in range(NCORES):
        hs = hidden_states[c * BPC : (c + 1) * BPC]  # [2, S, C]
        xt = np.ascontiguousarray(hs.transpose(0, 2, 1)).astype(bf)
        enc = encoder_hidden_states[c * BPC : (c + 1) * BPC]  # [2, 77, 1024]
        et = np.empty((CENC, SENC2), np.float32)
        for b in range(BPC):
            et[:, b * SENC : (b + 1) * SENC] = enc[b].T
        in_maps.append(
            {
                "xt": xt,
                "et": et.astype(bf),
                "wqt": wqt,
                "wkt": wkt,
                "wvt": wvt,
                "wot": wot,
            }
        )
    return in_maps


def kernel(hidden_states, encoder_hidden_states, Wq, Wk, Wv, Wo, bo,
           q_down, q_up, k_down, k_up, v_down, v_up, o_down, o_up):
    nc = _get_nc()
    in_maps = make_in_maps(
        hidden_states, encoder_hidden_states, Wq, Wk, Wv, Wo,
        q_down, q_up, k_down, k_up, v_down, v_up, o_down, o_up,
    )
    res = run_bass_kernel_spmd(nc, in_maps, list(range(NCORES)))
    out = np.concatenate(
        [np.asarray(res.results[c]["out"]).astype(np.float32) for c in range(NCORES)],
        axis=0,
    )
    out = out + bo.astype(np.float32)[None, None, :]
    return out.astype(np.float32)


# revision 14
# speedup vs baseline: 1.3252x; 1.3252x over previous
"""Trainium2 Bass kernel for LoRACrossAttnProcessor (v4, bf16, PE-continuity).

Strategy (changes vs v3):
- Q-proj(0) is the FIRST PE phase: it can start once x(b0,st0) + the
  first 256-col wq chunk land (~2MB) instead of waiting for wk+et+wq+x
  (~6MB).  K proj runs after Q0, its weights stream in under Q0's 46us
  of PE work with zero stall.
- Softmax normalization is applied to exps BEFORE the AV matmuls:
  sum-exp (PE) -> recip (DVE, f32) -> bf16 row -> partition-broadcast
  to 77 rows (GPSIMD) -> in-place bf16 multiply (DVE fast mode).  The
  whole chain runs under q_proj(1)/o_proj(0) PE cover.  AV evacuation
  is then a plain PSUM->SBUF cast, alternating DVE/ACT, with PSUM
  banks alternating psV/psSE, so the AV phase never starves.
- scores/softmax phases hoisted ahead of the big GEMMs; within a
  phase all 8 heads' score matmuls run before the 8 sum-exp matmuls.
  PE stalls are doubly expensive on TRN2: the PE clock drops to
  1.2GHz after an idle and needs 3us of continuous work to re-ramp.
- V projection over plain 1280 cols with scatter-evac into the
  zero-padded (head,tile)-pair layout.
- Fine-grained DMAs issued in consumption order; o_proj output DMA
  per 512-col chunk; evacuations alternate DVE/ACT.

Numerics: host folds LoRA exactly (f64), everything bf16 on device,
f32 PSUM.  (fp8 DoubleRow was measured at 2x bf16 MACs -- the 3-term
error-compensated scheme would be 1.4x slower than bf16, so not used.)
"""

import numpy as np
from contextlib import ExitStack

import ml_dtypes

import concourse.bass as bass
import concourse.mybir as mybir
import concourse.tile as tile
from concourse import bacc, bass_isa
from concourse.bass_utils import run_bass_kernel_spmd

F32 = mybir.dt.float32
BF16 = mybir.dt.bfloat16
AF = mybir.ActivationFunctionType
MULT = mybir.AluOpType.mult

H = 8
B, S, C = 16, 1024, 1280
SENC, CENC = 77, 1024
D = C // H  # 160
NCORES = 8
BPC = B // NCORES  # 2 batches per core
P = 128
NCI_Q = C // P  # 10 contraction tiles for Q/O proj
NCI_KV = CENC // P  # 8 contraction tiles for K/V proj
NCO = C // P  # 10 output-channel tiles
NST = S // 512  # 2 seq chunks of 512
SENC2 = 2 * SENC  # 154
ATTN_SCALE = 1.0 / float(np.sqrt(D))
OCHUNKS = [(0, 512), (512, 512), (1024, 256)]

# (head, tile) pairs: head h covers channels [160h, 160h+160); tile t covers
# [128t, 128t+128). Each pair gets one 128-col slot in the vm layout.
PAIRS = []
for _h in range(H):
    for _t in range(NCO):
        lo = max(D * _h, P * _t)
        hi = min(D * _h + D, P * _t + P)
        if lo < hi:
            PAIRS.append((_h, _t, lo, hi))
NPAIR = len(PAIRS)  # 16
PAIRS_OF_TILE = {t: [i for i, p in enumerate(PAIRS) if p[1] == t] for t in range(NCO)}
TILES_OF_HEAD = {h: sorted({p[1] for p in PAIRS if p[0] == h}) for h in range(H)}


def aligned_ranges(r0, r1):
    """Decompose [r0, r1) (within one 128 tile) into blocks of size 32/64/128
    with offset % size == 0 (SBUF partition-access alignment rule)."""
    out = []
    g = r0
    while g < r1:
        s = 128
        while s > r1 - g or g % s != 0:
            s //= 2
        out.append((g, s))
        g += s
    return out


def build():
    nc = bacc.Bacc("TRN2", target_bir_lowering=False, debug=False)
    xt_d = nc.dram_tensor("xt", [BPC, C, S], BF16, kind="ExternalInput")
    et_d = nc.dram_tensor("et", [CENC, SENC2], BF16, kind="ExternalInput")
    wqt_d = nc.dram_tensor("wqt", [C, C], BF16, kind="ExternalInput")
    wkt_d = nc.dram_tensor("wkt", [CENC, C], BF16, kind="ExternalInput")
    wvt_d = nc.dram_tensor("wvt", [CENC, C], BF16, kind="ExternalInput")
    wot_d = nc.dram_tensor("wot", [C, C], BF16, kind="ExternalInput")
    out_d = nc.dram_tensor("out", [BPC, S, C], BF16, kind="ExternalOutput")

    with tile.TileContext(nc) as tc, ExitStack() as ctx:
        big = ctx.enter_context(tc.tile_pool(name="big", bufs=4))
        wpool = ctx.enter_context(tc.tile_pool(name="wpool", bufs=1))
        persist = ctx.enter_context(tc.tile_pool(name="persist", bufs=1))
        expnp = ctx.enter_context(tc.tile_pool(name="expnp", bufs=2))
        smallp = ctx.enter_context(tc.tile_pool(name="smallp", bufs=4))
        stag = ctx.enter_context(tc.tile_pool(name="stag", bufs=2))
        psA = ctx.enter_context(tc.tile_pool(name="psA", bufs=3, space="PSUM"))
        psSE = ctx.enter_context(tc.tile_pool(name="psSE", bufs=3, space="PSUM"))
        psV = ctx.enter_context(tc.tile_pool(name="psV", bufs=2, space="PSUM"))

        # ---- DMAs in strict consumption order ----
        # Q0 feed first: x(b0,st0) per-ci + wq in 256-col chunks.
        x_s = [None] * BPC
        for b in range(BPC):
            x_s[b] = big.tile([P, NCI_Q, S], BF16, tag="big", name=f"xt{b}")
        wq_s = wpool.tile([P, NCI_Q, C], BF16, tag="wB", name="wq_s")

        def dma_x(b, st):
            sl = slice(st * 512, st * 512 + 512)
            for ci in range(NCI_Q):
                nc.sync.dma_start(
                    out=x_s[b][:, ci, sl],
                    in_=xt_d.ap()[b, ci * P : (ci + 1) * P, sl],
                )

        # first Q matmul needs only wq cols 0:128 + x(b0,st0,ci=0): 0.46MB
        def dma_wq(ch, cn):
            nc.sync.dma_start(
                out=wq_s[:, :, ch : ch + cn],
                in_=wqt_d.ap()[:, ch : ch + cn].rearrange(
                    "(ci p) c -> p ci c", p=P
                ),
            )

        def dma_x1(b, st, cis):
            sl = slice(st * 512, st * 512 + 512)
            for ci in cis:
                nc.sync.dma_start(
                    out=x_s[b][:, ci, sl],
                    in_=xt_d.ap()[b, ci * P : (ci + 1) * P, sl],
                )

        dma_wq(0, 128)
        dma_x1(0, 0, range(0, 7))
        dma_wq(128, 256)
        dma_x1(0, 0, range(7, 10))
        dma_wq(384, 256)
        dma_x1(0, 1, range(0, 5))
        dma_wq(640, 256)
        dma_x1(0, 1, range(5, 10))
        dma_wq(896, 256)
        dma_wq(1152, 128)

        # K feed (consumed ~+46us), then x(b1) (Q1 ~+60us), wv (V ~+105us),
        # wo (O0 ~+135us; shares wk's slot -> also waits on K proj reads).
        et_s = persist.tile([P, NCI_KV, SENC2], BF16, tag="et")
        wk_s = wpool.tile([P, NCI_KV, C], BF16, tag="wA")
        for ci in range(NCI_KV):
            nc.sync.dma_start(
                out=et_s[:, ci, :], in_=et_d.ap()[ci * P : (ci + 1) * P, :]
            )
            nc.sync.dma_start(
                out=wk_s[:, ci, :], in_=wkt_d.ap()[ci * P : (ci + 1) * P, :]
            )
        ones77 = persist.tile([SENC, 1], BF16, tag="ones77")
        nc.vector.memset(ones77, 1.0)
        dma_x(1, 0)
        dma_x(1, 1)
        wv_s = wpool.tile([P, NCI_KV, C], BF16, tag="wC", name="wv_s")
        for ci in range(NCI_KV):
            nc.sync.dma_start(
                out=wv_s[:, ci, :], in_=wvt_d.ap()[ci * P : (ci + 1) * P, :]
            )

        qt = [None] * BPC
        at = [None] * BPC
        vm = [None] * BPC
        kt_r = []
        exps_w = {}

        def q_proj(b, tiles):
            if qt[b] is None:
                qt[b] = big.tile([P, NCO, S], BF16, tag="big", name=f"qt{b}")
            for st, co in tiles:
                    sl = slice(st * 512, st * 512 + 512)
                    ps = psA.tile([P, 512], F32, tag="ps")
                    for ci in range(NCI_Q):
                        nc.tensor.matmul(
                            ps,
                            wq_s[:, ci, co * P : (co + 1) * P],
                            x_s[b][:, ci, sl],
                            start=(ci == 0),
                            stop=(ci == NCI_Q - 1),
                        )
                    nc.scalar.copy(out=qt[b][:, co, sl], in_=ps)

        def k_proj():
            for t in range(NCO):
                ps = psA.tile([P, 512], F32, tag="ps")
                for ci in range(NCI_KV):
                    nc.tensor.matmul(
                        ps[:, :SENC2],
                        wk_s[:, ci, t * P : (t + 1) * P],
                        et_s[:, ci, :],
                        start=(ci == 0),
                        stop=(ci == NCI_KV - 1),
                    )
                kte = persist.tile([P, SENC2], BF16, tag=f"kte{t}", name=f"kte{t}")
                kto = persist.tile([P, SENC2], BF16, tag=f"kto{t}", name=f"kto{t}")
                nc.vector.memset(kte, 0.0)
                nc.vector.memset(kto, 0.0)
                for h in range(H):
                    r0 = max(D * h, P * t)
                    r1 = min(D * h + D, P * t + P)
                    if r0 >= r1:
                        continue
                    dst = kte if h % 2 == 0 else kto
                    for o, sz in aligned_ranges(r0 - P * t, r1 - P * t):
                        nc.vector.tensor_copy(
                            out=dst[o : o + sz, :], in_=ps[o : o + sz, :SENC2]
                        )
                kt_r.append((kte, kto))

        def v_proj(b):
            # plain V proj (1280 cols in 3 chunks), scatter-evac into the
            # zero-padded (head,tile)-pair layout vm[b] [77, 16, 128]
            vm[b] = persist.tile([SENC, NPAIR, P], BF16, tag=f"vm{b}", name=f"vm{b}")
            for pi, (h, t, lo, hi) in enumerate(PAIRS):
                a, z = lo - P * t, hi - P * t
                if a > 0:
                    nc.gpsimd.memset(vm[b][:, pi, 0:a], 0.0)
                if z < P:
                    nc.gpsimd.memset(vm[b][:, pi, z:P], 0.0)
            bsl = slice(b * SENC, (b + 1) * SENC)
            k = 0
            for j, (c0, cn) in enumerate(OCHUNKS):
                ps = (psV if j % 2 == 0 else psSE).tile([P, 512], F32, tag="ps")
                for ci in range(NCI_KV):
                    nc.tensor.matmul(
                        ps[:SENC, :cn],
                        et_s[:, ci, bsl],
                        wv_s[:, ci, c0 : c0 + cn],
                        start=(ci == 0),
                        stop=(ci == NCI_KV - 1),
                    )
                for pi, (h, t, lo, hi) in enumerate(PAIRS):
                    if lo >= c0 + cn or hi <= c0:
                        continue
                    if k % 2 == 0:
                        nc.scalar.copy(
                            out=vm[b][:, pi, lo - P * t : hi - P * t],
                            in_=ps[:SENC, lo - c0 : hi - c0],
                        )
                    else:
                        nc.vector.tensor_copy(
                            out=vm[b][:, pi, lo - P * t : hi - P * t],
                            in_=ps[:SENC, lo - c0 : hi - c0],
                        )
                    k += 1

        def sc_part(b, st):
            # scores + exp for all 8 heads of one 512-query window
            bsl = slice(b * SENC, (b + 1) * SENC)
            sl = slice(st * 512, st * 512 + 512)
            exps = expnp.tile([SENC, H, 512], BF16, tag="expn")
            exps_w[(b, st)] = exps
            for h in range(H):
                tiles = TILES_OF_HEAD[h]
                ps_s = psSE.tile([SENC, 512], F32, tag="ps")
                for i, t in enumerate(tiles):
                    nc.tensor.matmul(
                        ps_s,
                        kt_r[t][h % 2][:, bsl],
                        qt[b][:, t, sl],
                        start=(i == 0),
                        stop=(i == len(tiles) - 1),
                    )
                nc.scalar.activation(
                    out=exps[:, h, :], in_=ps_s, func=AF.Exp, scale=ATTN_SCALE
                )

        def se_part(b, st):
            # per head: sum-exp (PE) -> recip (DVE f32) -> bf16 row ->
            # partition-broadcast (GPSIMD) -> in-place exps *= 1/Z (DVE).
            # All normalize work drains under following q/o_proj PE cover.
            exps = exps_w[(b, st)]
            for h in range(H):
                ps_se = psSE.tile([SENC, 512], F32, tag="ps", name="ps_se")
                nc.tensor.matmul(
                    ps_se[0:1, :], ones77, exps[:, h, :], start=True, stop=True
                )
                nc.vector.reciprocal_approx_fast(
                    out=ps_se[0:1, :], in_=ps_se[0:1, :]
                )
                recb = smallp.tile([1, 512], BF16, tag="recb")
                nc.vector.tensor_copy(out=recb, in_=ps_se[0:1, :])
                zb = smallp.tile([SENC, 512], BF16, tag="zb")
                nc.gpsimd.partition_broadcast(zb, recb)
                nc.vector.tensor_tensor(
                    out=exps[:, h, :], in0=exps[:, h, :], in1=zb, op=MULT
                )

        def av_phase(b, st):
            if at[b] is None:
                at[b] = big.tile([P, NCO, S], BF16, tag="big", name=f"at{b}")
            sl = slice(st * 512, st * 512 + 512)
            exps = exps_w.pop((b, st))
            for t in range(NCO):
                pairs = PAIRS_OF_TILE[t]
                ps_av = (psV if t % 2 == 0 else psSE).tile([P, 512], F32, tag="ps")
                for i, pi in enumerate(pairs):
                    ph = PAIRS[pi][0]
                    nc.tensor.matmul(
                        ps_av,
                        vm[b][:, pi, :],
                        exps[:, ph, :],
                        start=(i == 0),
                        stop=(i == len(pairs) - 1),
                    )
                if t % 2 == 0:
                    nc.vector.tensor_copy(out=at[b][:, t, sl], in_=ps_av)
                else:
                    nc.scalar.copy(out=at[b][:, t, sl], in_=ps_av)

        def o_proj(b, stiles):
            for stile in stiles:
                s0 = stile * P
                ost = stag.tile([P, C], BF16, tag="ost")
                for j, (c0, cn) in enumerate(OCHUNKS):
                    pso = psA.tile([P, 512], F32, tag="ps", name="pso")
                    for ci in range(NCI_Q):
                        nc.tensor.matmul(
                            pso[:, :cn],
                            at[b][:, ci, s0 : s0 + P],
                            wo_s[:, ci, c0 : c0 + cn],
                            start=(ci == 0),
                            stop=(ci == NCI_Q - 1),
                        )
                    if j % 2 == 0:
                        nc.vector.tensor_copy(
                            out=ost[:, c0 : c0 + cn], in_=pso[:, :cn]
                        )
                    else:
                        nc.scalar.copy(out=ost[:, c0 : c0 + cn], in_=pso[:, :cn])
                    nc.sync.dma_start(
                        out=out_d.ap()[b, s0 : s0 + P, c0 : c0 + cn],
                        in_=ost[:, c0 : c0 + cn],
                    )

        QTILES = [(st, co) for st in range(NST) for co in range(NCO)]
        # PE issue order = PE execution order (in-order engine queues).
        q_proj(0, QTILES)
        k_proj()
        # wo reuses wk's slot; issue after k_proj so the WAR dep is clean.
        wo_s = wpool.tile([P, NCI_Q, C], BF16, tag="wA", name="wo_s")
        for ci in range(NCI_Q):
            nc.sync.dma_start(
                out=wo_s[:, ci, :], in_=wot_d.ap()[ci * P : (ci + 1) * P, :]
            )
        sc_part(0, 0)
        se_part(0, 0)
        sc_part(0, 1)
        se_part(0, 1)
        q_proj(1, QTILES)
        v_proj(0)
        v_proj(1)
        av_phase(0, 0)
        av_phase(0, 1)
        sc_part(1, 0)
        se_part(1, 0)
        sc_part(1, 1)
        se_part(1, 1)
        o_proj(0, range(8))
        av_phase(1, 0)
        av_phase(1, 1)
        o_proj(1, range(8))

    nc.compile()
    return nc


_NC_CACHE = []


def _get_nc():
    if not _NC_CACHE:
        _NC_CACHE.append(build())
    return _NC_CACHE[0]


def make_in_maps(hidden_states, encoder_hidden_states, Wq, Wk, Wv, Wo,
                 q_down, q_up, k_down, k_up, v_down, v_up, o_down, o_up):
    bf = ml_dtypes.bfloat16
    wq = (Wq.astype(np.float64) + q_up.astype(np.float64) @ q_down.astype(np.float64))
    wk = (Wk.astype(np.float64) + k_up.astype(np.float64) @ k_down.astype(np.float64))
    wv = (Wv.astype(np.float64) + v_up.astype(np.float64) @ v_down.astype(np.float64))
    wo = (Wo.astype(np.float64) + o_up.astype(np.float64) @ o_down.astype(np.float64))
    wqt = np.ascontiguousarray(wq.T).astype(bf)
    wkt = np.ascontiguousarray(wk.T).astype(bf)
    wvt = np.ascontiguousarray(wv.T).astype(bf)
    wot = np.ascontiguousarray(wo.T).astype(bf)

    in_maps = []
    for c in range(NCORES):
        hs = hidden_states[c * BPC : (c + 1) * BPC]  # [2, S, C]
        xt = np.ascontiguousarray(hs.transpose(0, 2, 1)).astype(bf)
        enc = encoder_hidden_states[c * BPC : (c + 1) * BPC]  # [2, 77, 1024]
        et = np.empty((CENC, SENC2), np.float32)
        for b in range(BPC):
            et[:, b * SENC : (b + 1) * SENC] = enc[b].T
        in_maps.append(
            {
                "xt": xt,
                "et": et.astype(bf),
                "wqt": wqt,
                "wkt": wkt,
                "wvt": wvt,
                "wot": wot,
            }
        )
    return in_maps


def kernel(hidden_states, encoder_hidden_states, Wq, Wk, Wv, Wo, bo,
           q_down, q_up, k_down, k_up, v_down, v_up, o_down, o_up):
    nc = _get_nc()
    in_maps = make_in_maps(
        hidden_states, encoder_hidden_states, Wq, Wk, Wv, Wo,
        q_down, q_up, k_down, k_up, v_down, v_up, o_down, o_up,
    )
    res = run_bass_kernel_spmd(nc, in_maps, list(range(NCORES)))
    out = np.concatenate(
        [np.asarray(res.results[c]["out"]).astype(np.float32) for c in range(NCORES)],
        axis=0,
    )
    out = out + bo.astype(np.float32)[None, None, :]
    return out.astype(np.float32)


# revision 15
# speedup vs baseline: 1.3259x; 1.0005x over previous
"""Trainium2 Bass kernel for LoRACrossAttnProcessor (v4, bf16, PE-continuity).

Strategy (changes vs v3):
- Q-proj(0) is the FIRST PE phase: it can start once x(b0,st0) + the
  first 256-col wq chunk land (~2MB) instead of waiting for wk+et+wq+x
  (~6MB).  K proj runs after Q0, its weights stream in under Q0's 46us
  of PE work with zero stall.
- Softmax normalization is applied to exps BEFORE the AV matmuls:
  sum-exp (PE) -> recip (DVE, f32) -> bf16 row -> partition-broadcast
  to 77 rows (GPSIMD) -> in-place bf16 multiply (DVE fast mode).  The
  whole chain runs under q_proj(1)/o_proj(0) PE cover.  AV evacuation
  is then a plain PSUM->SBUF cast, alternating DVE/ACT, with PSUM
  banks alternating psV/psSE, so the AV phase never starves.
- scores/softmax phases hoisted ahead of the big GEMMs; within a
  phase all 8 heads' score matmuls run before the 8 sum-exp matmuls.
  PE stalls are doubly expensive on TRN2: the PE clock drops to
  1.2GHz after an idle and needs 3us of continuous work to re-ramp.
- V projection over plain 1280 cols with scatter-evac into the
  zero-padded (head,tile)-pair layout.
- Fine-grained DMAs issued in consumption order; o_proj output DMA
  per 512-col chunk; evacuations alternate DVE/ACT.

Numerics: host folds LoRA exactly (f64), everything bf16 on device,
f32 PSUM.  (fp8 DoubleRow was measured at 2x bf16 MACs -- the 3-term
error-compensated scheme would be 1.4x slower than bf16, so not used.)
"""

import numpy as np
from contextlib import ExitStack

import ml_dtypes

import concourse.bass as bass
import concourse.mybir as mybir
import concourse.tile as tile
from concourse import bacc, bass_isa
from concourse.bass_utils import run_bass_kernel_spmd

F32 = mybir.dt.float32
BF16 = mybir.dt.bfloat16
AF = mybir.ActivationFunctionType
MULT = mybir.AluOpType.mult

H = 8
B, S, C = 16, 1024, 1280
SENC, CENC = 77, 1024
D = C // H  # 160
NCORES = 8
BPC = B // NCORES  # 2 batches per core
P = 128
NCI_Q = C // P  # 10 contraction tiles for Q/O proj
NCI_KV = CENC // P  # 8 contraction tiles for K/V proj
NCO = C // P  # 10 output-channel tiles
NST = S // 512  # 2 seq chunks of 512
SENC2 = 2 * SENC  # 154
ATTN_SCALE = 1.0 / float(np.sqrt(D))
OCHUNKS = [(0, 512), (512, 512), (1024, 256)]

# (head, tile) pairs: head h covers channels [160h, 160h+160); tile t covers
# [128t, 128t+128). Each pair gets one 128-col slot in the vm layout.
PAIRS = []
for _h in range(H):
    for _t in range(NCO):
        lo = max(D * _h, P * _t)
        hi = min(D * _h + D, P * _t + P)
        if lo < hi:
            PAIRS.append((_h, _t, lo, hi))
NPAIR = len(PAIRS)  # 16
PAIRS_OF_TILE = {t: [i for i, p in enumerate(PAIRS) if p[1] == t] for t in range(NCO)}
TILES_OF_HEAD = {h: sorted({p[1] for p in PAIRS if p[0] == h}) for h in range(H)}


def aligned_ranges(r0, r1):
    """Decompose [r0, r1) (within one 128 tile) into blocks of size 32/64/128
    with offset % size == 0 (SBUF partition-access alignment rule)."""
    out = []
    g = r0
    while g < r1:
        s = 128
        while s > r1 - g or g % s != 0:
            s //= 2
        out.append((g, s))
        g += s
    return out


def build():
    nc = bacc.Bacc("TRN2", target_bir_lowering=False, debug=False)
    xt_d = nc.dram_tensor("xt", [BPC, C, S], BF16, kind="ExternalInput")
    et_d = nc.dram_tensor("et", [CENC, SENC2], BF16, kind="ExternalInput")
    wqt_d = nc.dram_tensor("wqt", [C, C], BF16, kind="ExternalInput")
    wkt_d = nc.dram_tensor("wkt", [CENC, C], BF16, kind="ExternalInput")
    wvt_d = nc.dram_tensor("wvt", [CENC, C], BF16, kind="ExternalInput")
    wot_d = nc.dram_tensor("wot", [C, C], BF16, kind="ExternalInput")
    out_d = nc.dram_tensor("out", [BPC, S, C], BF16, kind="ExternalOutput")

    with tile.TileContext(nc) as tc, ExitStack() as ctx:
        big = ctx.enter_context(tc.tile_pool(name="big", bufs=4))
        wpool = ctx.enter_context(tc.tile_pool(name="wpool", bufs=1))
        persist = ctx.enter_context(tc.tile_pool(name="persist", bufs=1))
        expnp = ctx.enter_context(tc.tile_pool(name="expnp", bufs=2))
        smallp = ctx.enter_context(tc.tile_pool(name="smallp", bufs=4))
        stag = ctx.enter_context(tc.tile_pool(name="stag", bufs=2))
        psA = ctx.enter_context(tc.tile_pool(name="psA", bufs=3, space="PSUM"))
        psSE = ctx.enter_context(tc.tile_pool(name="psSE", bufs=3, space="PSUM"))
        psV = ctx.enter_context(tc.tile_pool(name="psV", bufs=2, space="PSUM"))

        # ---- DMAs in strict consumption order ----
        # Q0 feed first: x(b0,st0) per-ci + wq in 256-col chunks.
        x_s = [None] * BPC
        for b in range(BPC):
            x_s[b] = big.tile([P, NCI_Q, S], BF16, tag="big", name=f"xt{b}")
        wq_s = wpool.tile([P, NCI_Q, C], BF16, tag="wB", name="wq_s")

        def dma_x(b, st):
            sl = slice(st * 512, st * 512 + 512)
            for ci in range(NCI_Q):
                nc.sync.dma_start(
                    out=x_s[b][:, ci, sl],
                    in_=xt_d.ap()[b, ci * P : (ci + 1) * P, sl],
                )

        # first Q matmul needs only wq cols 0:128 + x(b0,st0,ci=0): 0.46MB
        def dma_wq(ch, cn):
            nc.sync.dma_start(
                out=wq_s[:, :, ch : ch + cn],
                in_=wqt_d.ap()[:, ch : ch + cn].rearrange(
                    "(ci p) c -> p ci c", p=P
                ),
            )

        def dma_x1(b, st, cis):
            sl = slice(st * 512, st * 512 + 512)
            for ci in cis:
                nc.sync.dma_start(
                    out=x_s[b][:, ci, sl],
                    in_=xt_d.ap()[b, ci * P : (ci + 1) * P, sl],
                )

        dma_wq(0, 128)
        dma_x1(0, 0, range(0, 10))
        dma_wq(128, 256)
        dma_wq(384, 256)
        dma_wq(640, 256)
        dma_x1(0, 1, range(0, 5))
        dma_wq(896, 256)
        dma_wq(1152, 128)
        dma_x1(0, 1, range(5, 10))

        # K feed (consumed ~+46us), then x(b1) (Q1 ~+60us), wv (V ~+105us),
        # wo (O0 ~+135us; shares wk's slot -> also waits on K proj reads).
        et_s = persist.tile([P, NCI_KV, SENC2], BF16, tag="et")
        wk_s = wpool.tile([P, NCI_KV, C], BF16, tag="wA")
        for ci in range(NCI_KV):
            nc.sync.dma_start(
                out=et_s[:, ci, :], in_=et_d.ap()[ci * P : (ci + 1) * P, :]
            )
            nc.sync.dma_start(
                out=wk_s[:, ci, :], in_=wkt_d.ap()[ci * P : (ci + 1) * P, :]
            )
        ones77 = persist.tile([SENC, 1], BF16, tag="ones77")
        nc.vector.memset(ones77, 1.0)
        dma_x(1, 0)
        dma_x(1, 1)
        wv_s = wpool.tile([P, NCI_KV, C], BF16, tag="wC", name="wv_s")
        for ci in range(NCI_KV):
            nc.sync.dma_start(
                out=wv_s[:, ci, :], in_=wvt_d.ap()[ci * P : (ci + 1) * P, :]
            )

        qt = [None] * BPC
        at = [None] * BPC
        vm = [None] * BPC
        kt_r = []
        exps_w = {}

        def q_proj(b, tiles):
            if qt[b] is None:
                qt[b] = big.tile([P, NCO, S], BF16, tag="big", name=f"qt{b}")
            for st, co in tiles:
                    sl = slice(st * 512, st * 512 + 512)
                    ps = psA.tile([P, 512], F32, tag="ps")
                    for ci in range(NCI_Q):
                        nc.tensor.matmul(
                            ps,
                            wq_s[:, ci, co * P : (co + 1) * P],
                            x_s[b][:, ci, sl],
                            start=(ci == 0),
                            stop=(ci == NCI_Q - 1),
                        )
                    nc.scalar.copy(out=qt[b][:, co, sl], in_=ps)

        def k_proj():
            for t in range(NCO):
                ps = psA.tile([P, 512], F32, tag="ps")
                for ci in range(NCI_KV):
                    nc.tensor.matmul(
                        ps[:, :SENC2],
                        wk_s[:, ci, t * P : (t + 1) * P],
                        et_s[:, ci, :],
                        start=(ci == 0),
                        stop=(ci == NCI_KV - 1),
                    )
                kte = persist.tile([P, SENC2], BF16, tag=f"kte{t}", name=f"kte{t}")
                kto = persist.tile([P, SENC2], BF16, tag=f"kto{t}", name=f"kto{t}")
                nc.vector.memset(kte, 0.0)
                nc.vector.memset(kto, 0.0)
                for h in range(H):
                    r0 = max(D * h, P * t)
                    r1 = min(D * h + D, P * t + P)
                    if r0 >= r1:
                        continue
                    dst = kte if h % 2 == 0 else kto
                    for o, sz in aligned_ranges(r0 - P * t, r1 - P * t):
                        nc.vector.tensor_copy(
                            out=dst[o : o + sz, :], in_=ps[o : o + sz, :SENC2]
                        )
                kt_r.append((kte, kto))

        def v_proj(b):
            # plain V proj (1280 cols in 3 chunks), scatter-evac into the
            # zero-padded (head,tile)-pair layout vm[b] [77, 16, 128]
            vm[b] = persist.tile([SENC, NPAIR, P], BF16, tag=f"vm{b}", name=f"vm{b}")
            for pi, (h, t, lo, hi) in enumerate(PAIRS):
                a, z = lo - P * t, hi - P * t
                if a > 0:
                    nc.gpsimd.memset(vm[b][:, pi, 0:a], 0.0)
                if z < P:
                    nc.gpsimd.memset(vm[b][:, pi, z:P], 0.0)
            bsl = slice(b * SENC, (b + 1) * SENC)
            k = 0
            for j, (c0, cn) in enumerate(OCHUNKS):
                ps = (psV if j % 2 == 0 else psSE).tile([P, 512], F32, tag="ps")
                for ci in range(NCI_KV):
                    nc.tensor.matmul(
                        ps[:SENC, :cn],
                        et_s[:, ci, bsl],
                        wv_s[:, ci, c0 : c0 + cn],
                        start=(ci == 0),
                        stop=(ci == NCI_KV - 1),
                    )
                for pi, (h, t, lo, hi) in enumerate(PAIRS):
                    if lo >= c0 + cn or hi <= c0:
                        continue
                    if k % 2 == 0:
                        nc.scalar.copy(
                            out=vm[b][:, pi, lo - P * t : hi - P * t],
                            in_=ps[:SENC, lo - c0 : hi - c0],
                        )
                    else:
                        nc.vector.tensor_copy(
                            out=vm[b][:, pi, lo - P * t : hi - P * t],
                            in_=ps[:SENC, lo - c0 : hi - c0],
                        )
                    k += 1

        def sc_part(b, st):
            # scores + exp for all 8 heads of one 512-query window
            bsl = slice(b * SENC, (b + 1) * SENC)
            sl = slice(st * 512, st * 512 + 512)
            exps = expnp.tile([SENC, H, 512], BF16, tag="expn")
            exps_w[(b, st)] = exps
            for h in range(H):
                tiles = TILES_OF_HEAD[h]
                ps_s = psSE.tile([SENC, 512], F32, tag="ps")
                for i, t in enumerate(tiles):
                    nc.tensor.matmul(
                        ps_s,
                        kt_r[t][h % 2][:, bsl],
                        qt[b][:, t, sl],
                        start=(i == 0),
                        stop=(i == len(tiles) - 1),
                    )
                nc.scalar.activation(
                    out=exps[:, h, :], in_=ps_s, func=AF.Exp, scale=ATTN_SCALE
                )

        def se_part(b, st):
            # per head: sum-exp (PE) -> recip (DVE f32) -> bf16 row ->
            # partition-broadcast (GPSIMD) -> in-place exps *= 1/Z (DVE).
            # All normalize work drains under following q/o_proj PE cover.
            exps = exps_w[(b, st)]
            for h in range(H):
                ps_se = psSE.tile([SENC, 512], F32, tag="ps", name="ps_se")
                nc.tensor.matmul(
                    ps_se[0:1, :], ones77, exps[:, h, :], start=True, stop=True
                )
                nc.vector.reciprocal_approx_fast(
                    out=ps_se[0:1, :], in_=ps_se[0:1, :]
                )
                recb = smallp.tile([1, 512], BF16, tag="recb")
                nc.vector.tensor_copy(out=recb, in_=ps_se[0:1, :])
                zb = smallp.tile([SENC, 512], BF16, tag="zb")
                nc.gpsimd.partition_broadcast(zb, recb)
                nc.vector.tensor_tensor(
                    out=exps[:, h, :], in0=exps[:, h, :], in1=zb, op=MULT
                )

        def av_phase(b, st):
            if at[b] is None:
                at[b] = big.tile([P, NCO, S], BF16, tag="big", name=f"at{b}")
            sl = slice(st * 512, st * 512 + 512)
            exps = exps_w.pop((b, st))
            for t in range(NCO):
                pairs = PAIRS_OF_TILE[t]
                ps_av = (psV if t % 2 == 0 else psSE).tile([P, 512], F32, tag="ps")
                for i, pi in enumerate(pairs):
                    ph = PAIRS[pi][0]
                    nc.tensor.matmul(
                        ps_av,
                        vm[b][:, pi, :],
                        exps[:, ph, :],
                        start=(i == 0),
                        stop=(i == len(pairs) - 1),
                    )
                sla = slice(st * 512, st * 512 + 256)
                slb = slice(st * 512 + 256, st * 512 + 512)
                nc.vector.tensor_copy(out=at[b][:, t, sla], in_=ps_av[:, :256])
                nc.scalar.copy(out=at[b][:, t, slb], in_=ps_av[:, 256:])

        def o_proj(b, stiles):
            for stile in stiles:
                s0 = stile * P
                ost = stag.tile([P, C], BF16, tag="ost")
                for j, (c0, cn) in enumerate(OCHUNKS):
                    pso = psA.tile([P, 512], F32, tag="ps", name="pso")
                    for ci in range(NCI_Q):
                        nc.tensor.matmul(
                            pso[:, :cn],
                            at[b][:, ci, s0 : s0 + P],
                            wo_s[:, ci, c0 : c0 + cn],
                            start=(ci == 0),
                            stop=(ci == NCI_Q - 1),
                        )
                    if j % 2 == 0:
                        nc.vector.tensor_copy(
                            out=ost[:, c0 : c0 + cn], in_=pso[:, :cn]
                        )
                    else:
                        nc.scalar.copy(out=ost[:, c0 : c0 + cn], in_=pso[:, :cn])
                    nc.sync.dma_start(
                        out=out_d.ap()[b, s0 : s0 + P, c0 : c0 + cn],
                        in_=ost[:, c0 : c0 + cn],
                    )

        QTILES = [(st, co) for st in range(NST) for co in range(NCO)]
        # PE issue order = PE execution order (in-order engine queues).
        q_proj(0, QTILES)
        k_proj()
        # wo reuses wk's slot; issue after k_proj so the WAR dep is clean.
        wo_s = wpool.tile([P, NCI_Q, C], BF16, tag="wA", name="wo_s")
        for ci in range(NCI_Q):
            nc.sync.dma_start(
                out=wo_s[:, ci, :], in_=wot_d.ap()[ci * P : (ci + 1) * P, :]
            )
        sc_part(0, 0)
        se_part(0, 0)
        sc_part(0, 1)
        se_part(0, 1)
        q_proj(1, QTILES)
        v_proj(0)
        v_proj(1)
        av_phase(0, 0)
        av_phase(0, 1)
        sc_part(1, 0)
        se_part(1, 0)
        sc_part(1, 1)
        se_part(1, 1)
        o_proj(0, range(8))
        av_phase(1, 0)
        av_phase(1, 1)
        o_proj(1, range(8))

    nc.compile()
    return nc


_NC_CACHE = []


def _get_nc():
    if not _NC_CACHE:
        _NC_CACHE.append(build())
    return _NC_CACHE[0]


def make_in_maps(hidden_states, encoder_hidden_states, Wq, Wk, Wv, Wo,
                 q_down, q_up, k_down, k_up, v_down, v_up, o_down, o_up):
    bf = ml_dtypes.bfloat16
    wq = (Wq.astype(np.float64) + q_up.astype(np.float64) @ q_down.astype(np.float64))
    wk = (Wk.astype(np.float64) + k_up.astype(np.float64) @ k_down.astype(np.float64))
    wv = (Wv.astype(np.float64) + v_up.astype(np.float64) @ v_down.astype(np.float64))
    wo = (Wo.astype(np.float64) + o_up.astype(np.float64) @ o_down.astype(np.float64))
    wqt = np.ascontiguousarray(wq.T).astype(bf)
    wkt = np.ascontiguousarray(wk.T).astype(bf)
    wvt = np.ascontiguousarray(wv.T).astype(bf)
    wot = np.ascontiguousarray(wo.T).astype(bf)

    in_maps = []
    for c in range(NCORES):
        hs = hidden_states[c * BPC : (c + 1) * BPC]  # [2, S, C]
        xt = np.ascontiguousarray(hs.transpose(0, 2, 1)).astype(bf)
        enc = encoder_hidden_states[c * BPC : (c + 1) * BPC]  # [2, 77, 1024]
        et = np.empty((CENC, SENC2), np.float32)
        for b in range(BPC):
            et[:, b * SENC : (b + 1) * SENC] = enc[b].T
        in_maps.append(
            {
                "xt": xt,
                "et": et.astype(bf),
                "wqt": wqt,
                "wkt": wkt,
                "wvt": wvt,
                "wot": wot,
            }
        )
    return in_maps


def kernel(hidden_states, encoder_hidden_states, Wq, Wk, Wv, Wo, bo,
           q_down, q_up, k_down, k_up, v_down, v_up, o_down, o_up):
    nc = _get_nc()
    in_maps = make_in_maps(
        hidden_states, encoder_hidden_states, Wq, Wk, Wv, Wo,
        q_down, q_up, k_down, k_up, v_down, v_up, o_down, o_up,
    )
    res = run_bass_kernel_spmd(nc, in_maps, list(range(NCORES)))
    out = np.concatenate(
        [np.asarray(res.results[c]["out"]).astype(np.float32) for c in range(NCORES)],
        axis=0,
    )
    out = out + bo.astype(np.float32)[None, None, :]
    return out.astype(np.float32)


# revision 16
# speedup vs baseline: 1.3665x; 1.0307x over previous
"""Trainium2 Bass kernel for LoRACrossAttnProcessor (v4, bf16, PE-continuity).

Strategy (changes vs v3):
- Q-proj(0) is the FIRST PE phase: it can start once x(b0,st0) + the
  first 256-col wq chunk land (~2MB) instead of waiting for wk+et+wq+x
  (~6MB).  K proj runs after Q0, its weights stream in under Q0's 46us
  of PE work with zero stall.
- Softmax normalization is applied to exps BEFORE the AV matmuls:
  sum-exp (PE) -> recip (DVE, f32) -> bf16 row -> partition-broadcast
  to 77 rows (GPSIMD) -> in-place bf16 multiply (DVE fast mode).  The
  whole chain runs under q_proj(1)/o_proj(0) PE cover.  AV evacuation
  is then a plain PSUM->SBUF cast, alternating DVE/ACT, with PSUM
  banks alternating psV/psSE, so the AV phase never starves.
- scores/softmax phases hoisted ahead of the big GEMMs; within a
  phase all 8 heads' score matmuls run before the 8 sum-exp matmuls.
  PE stalls are doubly expensive on TRN2: the PE clock drops to
  1.2GHz after an idle and needs 3us of continuous work to re-ramp.
- V projection over plain 1280 cols with scatter-evac into the
  zero-padded (head,tile)-pair layout.
- Fine-grained DMAs issued in consumption order; o_proj output DMA
  per 512-col chunk; evacuations alternate DVE/ACT.

Numerics: host folds LoRA exactly (f64), everything bf16 on device,
f32 PSUM.  (fp8 DoubleRow was measured at 2x bf16 MACs -- the 3-term
error-compensated scheme would be 1.4x slower than bf16, so not used.)
"""

import numpy as np
from contextlib import ExitStack

import ml_dtypes

import concourse.bass as bass
import concourse.mybir as mybir
import concourse.tile as tile
from concourse import bacc, bass_isa
from concourse.bass_utils import run_bass_kernel_spmd

F32 = mybir.dt.float32
BF16 = mybir.dt.bfloat16
AF = mybir.ActivationFunctionType
MULT = mybir.AluOpType.mult

H = 8
B, S, C = 16, 1024, 1280
SENC, CENC = 77, 1024
D = C // H  # 160
NCORES = 8
BPC = B // NCORES  # 2 batches per core
P = 128
NCI_Q = C // P  # 10 contraction tiles for Q/O proj
NCI_KV = CENC // P  # 8 contraction tiles for K/V proj
NCO = C // P  # 10 output-channel tiles
NST = S // 512  # 2 seq chunks of 512
SENC2 = 2 * SENC  # 154
ATTN_SCALE = 1.0 / float(np.sqrt(D))
OCHUNKS = [(0, 512), (512, 512), (1024, 256)]

# (head, tile) pairs: head h covers channels [160h, 160h+160); tile t covers
# [128t, 128t+128). Each pair gets one 128-col slot in the vm layout.
PAIRS = []
for _h in range(H):
    for _t in range(NCO):
        lo = max(D * _h, P * _t)
        hi = min(D * _h + D, P * _t + P)
        if lo < hi:
            PAIRS.append((_h, _t, lo, hi))
NPAIR = len(PAIRS)  # 16
PAIRS_OF_TILE = {t: [i for i, p in enumerate(PAIRS) if p[1] == t] for t in range(NCO)}
TILES_OF_HEAD = {h: sorted({p[1] for p in PAIRS if p[0] == h}) for h in range(H)}


def aligned_ranges(r0, r1):
    """Decompose [r0, r1) (within one 128 tile) into blocks of size 32/64/128
    with offset % size == 0 (SBUF partition-access alignment rule)."""
    out = []
    g = r0
    while g < r1:
        s = 128
        while s > r1 - g or g % s != 0:
            s //= 2
        out.append((g, s))
        g += s
    return out


def build():
    nc = bacc.Bacc("TRN2", target_bir_lowering=False, debug=False)
    xt_d = nc.dram_tensor("xt", [BPC, C, S], BF16, kind="ExternalInput")
    et_d = nc.dram_tensor("et", [CENC, SENC2], BF16, kind="ExternalInput")
    wqt_d = nc.dram_tensor("wqt", [C, C], BF16, kind="ExternalInput")
    wkt_d = nc.dram_tensor("wkt", [CENC, C], BF16, kind="ExternalInput")
    wvt_d = nc.dram_tensor("wvt", [CENC, C], BF16, kind="ExternalInput")
    wot_d = nc.dram_tensor("wot", [C, C], BF16, kind="ExternalInput")
    out_d = nc.dram_tensor("out", [BPC, S, C], BF16, kind="ExternalOutput")

    with tile.TileContext(nc) as tc, ExitStack() as ctx:
        big = ctx.enter_context(tc.tile_pool(name="big", bufs=4))
        wpool = ctx.enter_context(tc.tile_pool(name="wpool", bufs=1))
        persist = ctx.enter_context(tc.tile_pool(name="persist", bufs=1))
        expnp = ctx.enter_context(tc.tile_pool(name="expnp", bufs=2))
        smallp = ctx.enter_context(tc.tile_pool(name="smallp", bufs=4))
        stag = ctx.enter_context(tc.tile_pool(name="stag", bufs=2))
        psA = ctx.enter_context(tc.tile_pool(name="psA", bufs=3, space="PSUM"))
        psSE = ctx.enter_context(tc.tile_pool(name="psSE", bufs=3, space="PSUM"))
        psV = ctx.enter_context(tc.tile_pool(name="psV", bufs=2, space="PSUM"))

        # ---- DMAs in strict consumption order ----
        # Q0 feed first: x(b0,st0) per-ci + wq in 256-col chunks.
        x_s = [None] * BPC
        for b in range(BPC):
            x_s[b] = big.tile([P, NCI_Q, S], BF16, tag="big", name=f"xt{b}")
        wq_s = wpool.tile([P, NCI_Q, C], BF16, tag="wB", name="wq_s")

        def dma_x(b, st):
            sl = slice(st * 512, st * 512 + 512)
            for ci in range(NCI_Q):
                nc.sync.dma_start(
                    out=x_s[b][:, ci, sl],
                    in_=xt_d.ap()[b, ci * P : (ci + 1) * P, sl],
                )

        # first Q matmul needs only wq cols 0:128 + x(b0,st0,ci=0): 0.46MB
        def dma_wq(ch, cn):
            nc.sync.dma_start(
                out=wq_s[:, :, ch : ch + cn],
                in_=wqt_d.ap()[:, ch : ch + cn].rearrange(
                    "(ci p) c -> p ci c", p=P
                ),
            )

        def dma_x1(b, st, cis):
            sl = slice(st * 512, st * 512 + 512)
            for ci in cis:
                nc.sync.dma_start(
                    out=x_s[b][:, ci, sl],
                    in_=xt_d.ap()[b, ci * P : (ci + 1) * P, sl],
                )

        dma_wq(0, 128)
        dma_x1(0, 0, range(0, 10))
        dma_wq(128, 256)
        dma_wq(384, 256)
        dma_wq(640, 256)
        dma_x1(0, 1, range(0, 5))
        dma_wq(896, 256)
        dma_wq(1152, 128)
        dma_x1(0, 1, range(5, 10))

        # K feed (consumed ~+46us), then x(b1) (Q1 ~+60us), wv (V ~+105us),
        # wo (O0 ~+135us; shares wk's slot -> also waits on K proj reads).
        et_s = persist.tile([P, NCI_KV, SENC2], BF16, tag="et")
        wk_s = wpool.tile([P, NCI_KV, C], BF16, tag="wA")
        for ci in range(NCI_KV):
            nc.sync.dma_start(
                out=et_s[:, ci, :], in_=et_d.ap()[ci * P : (ci + 1) * P, :]
            )
            nc.sync.dma_start(
                out=wk_s[:, ci, :], in_=wkt_d.ap()[ci * P : (ci + 1) * P, :]
            )
        ones77 = persist.tile([SENC, 1], BF16, tag="ones77")
        nc.vector.memset(ones77, 1.0)
        dma_x(1, 0)
        dma_x(1, 1)
        wv_s = wpool.tile([P, NCI_KV, C], BF16, tag="wC", name="wv_s")
        for ci in range(NCI_KV):
            nc.sync.dma_start(
                out=wv_s[:, ci, :], in_=wvt_d.ap()[ci * P : (ci + 1) * P, :]
            )

        qt = [None] * BPC
        at = [None] * BPC
        vm = [None] * BPC
        kt_r = []
        exps_w = {}

        def q_proj(b, tiles):
            if qt[b] is None:
                qt[b] = big.tile([P, NCO, S], BF16, tag="big", name=f"qt{b}")
            for st, co in tiles:
                    sl = slice(st * 512, st * 512 + 512)
                    ps = psA.tile([P, 512], F32, tag="ps")
                    for ci in range(NCI_Q):
                        nc.tensor.matmul(
                            ps,
                            wq_s[:, ci, co * P : (co + 1) * P],
                            x_s[b][:, ci, sl],
                            start=(ci == 0),
                            stop=(ci == NCI_Q - 1),
                        )
                    nc.scalar.copy(out=qt[b][:, co, sl], in_=ps)

        def k_proj():
            for t in range(NCO):
                ps = psA.tile([P, 512], F32, tag="ps")
                for ci in range(NCI_KV):
                    nc.tensor.matmul(
                        ps[:, :SENC2],
                        wk_s[:, ci, t * P : (t + 1) * P],
                        et_s[:, ci, :],
                        start=(ci == 0),
                        stop=(ci == NCI_KV - 1),
                    )
                kte = persist.tile([P, SENC2], BF16, tag=f"kte{t}", name=f"kte{t}")
                kto = persist.tile([P, SENC2], BF16, tag=f"kto{t}", name=f"kto{t}")
                nc.vector.memset(kte, 0.0)
                nc.vector.memset(kto, 0.0)
                for h in range(H):
                    r0 = max(D * h, P * t)
                    r1 = min(D * h + D, P * t + P)
                    if r0 >= r1:
                        continue
                    dst = kte if h % 2 == 0 else kto
                    for o, sz in aligned_ranges(r0 - P * t, r1 - P * t):
                        nc.vector.tensor_copy(
                            out=dst[o : o + sz, :], in_=ps[o : o + sz, :SENC2]
                        )
                kt_r.append((kte, kto))

        def v_proj(b):
            # plain V proj (1280 cols in 3 chunks), scatter-evac into the
            # zero-padded (head,tile)-pair layout vm[b] [77, 16, 128]
            vm[b] = persist.tile([SENC, NPAIR, P], BF16, tag=f"vm{b}", name=f"vm{b}")
            for pi, (h, t, lo, hi) in enumerate(PAIRS):
                a, z = lo - P * t, hi - P * t
                if a > 0:
                    nc.gpsimd.memset(vm[b][:, pi, 0:a], 0.0)
                if z < P:
                    nc.gpsimd.memset(vm[b][:, pi, z:P], 0.0)
            bsl = slice(b * SENC, (b + 1) * SENC)
            k = 0
            for j, (c0, cn) in enumerate(OCHUNKS):
                ps = (psV if j % 2 == 0 else psSE).tile([P, 512], F32, tag="ps")
                for ci in range(NCI_KV):
                    nc.tensor.matmul(
                        ps[:SENC, :cn],
                        et_s[:, ci, bsl],
                        wv_s[:, ci, c0 : c0 + cn],
                        start=(ci == 0),
                        stop=(ci == NCI_KV - 1),
                    )
                for pi, (h, t, lo, hi) in enumerate(PAIRS):
                    if lo >= c0 + cn or hi <= c0:
                        continue
                    if k % 2 == 0:
                        nc.scalar.copy(
                            out=vm[b][:, pi, lo - P * t : hi - P * t],
                            in_=ps[:SENC, lo - c0 : hi - c0],
                        )
                    else:
                        nc.vector.tensor_copy(
                            out=vm[b][:, pi, lo - P * t : hi - P * t],
                            in_=ps[:SENC, lo - c0 : hi - c0],
                        )
                    k += 1

        def sc_part(b, st):
            # scores + exp for all 8 heads of one 512-query window
            bsl = slice(b * SENC, (b + 1) * SENC)
            sl = slice(st * 512, st * 512 + 512)
            exps = expnp.tile([SENC, H, 512], BF16, tag="expn")
            exps_w[(b, st)] = exps
            for h in range(H):
                tiles = TILES_OF_HEAD[h]
                ps_s = psSE.tile([SENC, 512], F32, tag="ps")
                for i, t in enumerate(tiles):
                    nc.tensor.matmul(
                        ps_s,
                        kt_r[t][h % 2][:, bsl],
                        qt[b][:, t, sl],
                        start=(i == 0),
                        stop=(i == len(tiles) - 1),
                    )
                nc.scalar.activation(
                    out=exps[:, h, :], in_=ps_s, func=AF.Exp, scale=ATTN_SCALE
                )

        def se_part(b, st):
            # per head: sum-exp (PE) -> recip (DVE f32) -> bf16 row ->
            # partition-broadcast (GPSIMD) -> in-place exps *= 1/Z (DVE).
            # All normalize work drains under following q/o_proj PE cover.
            exps = exps_w[(b, st)]
            for h in range(H):
                ps_se = psSE.tile([SENC, 512], F32, tag="ps", name="ps_se")
                nc.tensor.matmul(
                    ps_se[0:1, :], ones77, exps[:, h, :], start=True, stop=True
                )
                nc.vector.reciprocal_approx_fast(
                    out=ps_se[0:1, :], in_=ps_se[0:1, :]
                )
                recb = smallp.tile([1, 512], BF16, tag="recb")
                nc.vector.tensor_copy(out=recb, in_=ps_se[0:1, :])
                zb = smallp.tile([SENC, 512], BF16, tag="zb")
                nc.gpsimd.partition_broadcast(zb, recb)
                nc.vector.tensor_tensor(
                    out=exps[:, h, :], in0=exps[:, h, :], in1=zb, op=MULT
                )

        def av_phase(b, st):
            if at[b] is None:
                at[b] = big.tile([P, NCO, S], BF16, tag="big", name=f"at{b}")
            sl = slice(st * 512, st * 512 + 512)
            exps = exps_w.pop((b, st))
            for t in range(NCO):
                pairs = PAIRS_OF_TILE[t]
                ps_av = (psV if t % 2 == 0 else psSE).tile([P, 512], F32, tag="ps")
                for i, pi in enumerate(pairs):
                    ph = PAIRS[pi][0]
                    nc.tensor.matmul(
                        ps_av,
                        vm[b][:, pi, :],
                        exps[:, ph, :],
                        start=(i == 0),
                        stop=(i == len(pairs) - 1),
                    )
                sla = slice(st * 512, st * 512 + 256)
                slb = slice(st * 512 + 256, st * 512 + 512)
                nc.vector.tensor_copy(out=at[b][:, t, sla], in_=ps_av[:, :256])
                nc.scalar.copy(out=at[b][:, t, slb], in_=ps_av[:, 256:])

        def o_proj(b, stiles):
            for stile in stiles:
                s0 = stile * P
                ost = stag.tile([P, C], BF16, tag="ost")
                for j, (c0, cn) in enumerate(OCHUNKS):
                    pso = psA.tile([P, 512], F32, tag="ps", name="pso")
                    for ci in range(NCI_Q):
                        nc.tensor.matmul(
                            pso[:, :cn],
                            at[b][:, ci, s0 : s0 + P],
                            wo_s[:, ci, c0 : c0 + cn],
                            start=(ci == 0),
                            stop=(ci == NCI_Q - 1),
                        )
                    nc.scalar.copy(out=ost[:, c0 : c0 + cn], in_=pso[:, :cn])
                    nc.sync.dma_start(
                        out=out_d.ap()[b, s0 : s0 + P, c0 : c0 + cn],
                        in_=ost[:, c0 : c0 + cn],
                    )

        QTILES = [(st, co) for st in range(NST) for co in range(NCO)]
        # PE issue order = PE execution order (in-order engine queues).
        q_proj(0, QTILES)
        k_proj()
        # wo reuses wk's slot; issue after k_proj so the WAR dep is clean.
        wo_s = wpool.tile([P, NCI_Q, C], BF16, tag="wA", name="wo_s")
        for ci in range(NCI_Q):
            nc.sync.dma_start(
                out=wo_s[:, ci, :], in_=wot_d.ap()[ci * P : (ci + 1) * P, :]
            )
        sc_part(0, 0)
        se_part(0, 0)
        sc_part(0, 1)
        se_part(0, 1)
        q_proj(1, QTILES)
        v_proj(0)
        v_proj(1)
        av_phase(0, 0)
        av_phase(0, 1)
        sc_part(1, 0)
        se_part(1, 0)
        sc_part(1, 1)
        se_part(1, 1)
        o_proj(0, range(8))
        av_phase(1, 0)
        av_phase(1, 1)
        o_proj(1, range(8))

    nc.compile()
    return nc


_NC_CACHE = []


def _get_nc():
    if not _NC_CACHE:
        _NC_CACHE.append(build())
    return _NC_CACHE[0]


def make_in_maps(hidden_states, encoder_hidden_states, Wq, Wk, Wv, Wo,
                 q_down, q_up, k_down, k_up, v_down, v_up, o_down, o_up):
    bf = ml_dtypes.bfloat16
    wq = (Wq.astype(np.float64) + q_up.astype(np.float64) @ q_down.astype(np.float64))
    wk = (Wk.astype(np.float64) + k_up.astype(np.float64) @ k_down.astype(np.float64))
    wv = (Wv.astype(np.float64) + v_up.astype(np.float64) @ v_down.astype(np.float64))
    wo = (Wo.astype(np.float64) + o_up.astype(np.float64) @ o_down.astype(np.float64))
    wqt = np.ascontiguousarray(wq.T).astype(bf)
    wkt = np.ascontiguousarray(wk.T).astype(bf)
    wvt = np.ascontiguousarray(wv.T).astype(bf)
    wot = np.ascontiguousarray(wo.T).astype(bf)

    in_maps = []
    for c in range(NCORES):
        hs = hidden_states[c * BPC : (c + 1) * BPC]  # [2, S, C]
        xt = np.ascontiguousarray(hs.transpose(0, 2, 1)).astype(bf)
        enc = encoder_hidden_states[c * BPC : (c + 1) * BPC]  # [2, 77, 1024]
        et = np.empty((CENC, SENC2), np.float32)
        for b in range(BPC):
            et[:, b * SENC : (b + 1) * SENC] = enc[b].T
        in_maps.append(
            {
                "xt": xt,
                "et": et.astype(bf),
                "wqt": wqt,
                "wkt": wkt,
                "wvt": wvt,
                "wot": wot,
            }
        )
    return in_maps


def kernel(hidden_states, encoder_hidden_states, Wq, Wk, Wv, Wo, bo,
           q_down, q_up, k_down, k_up, v_down, v_up, o_down, o_up):
    nc = _get_nc()
    in_maps = make_in_maps(
        hidden_states, encoder_hidden_states, Wq, Wk, Wv, Wo,
        q_down, q_up, k_down, k_up, v_down, v_up, o_down, o_up,
    )
    res = run_bass_kernel_spmd(nc, in_maps, list(range(NCORES)))
    out = np.concatenate(
        [np.asarray(res.results[c]["out"]).astype(np.float32) for c in range(NCORES)],
        axis=0,
    )
    out = out + bo.astype(np.float32)[None, None, :]
    return out.astype(np.float32)
